# revision 5
# baseline (speedup 1.0000x reference)
"""ELPH edge-aware GNN message passing on 8 Trainium2 NeuronCores.

Strategy (edge-parallel, per the sharding hint, with a dst-sort refinement):
  - Sort edges by destination and shard them so core c owns all edges whose
    dst lies in its 12500-node range. The per-device scatter-add then needs
    no all-reduce: each core aggregates only into its own node slice.
  - Within a core, edges are grouped into 128-node destination windows
    ("blocks"); each 128-edge chunk's scatter-add is one PSUM-accumulated
    matmul against a 0/1 selection matrix built on-device (iota == dst_local).
  - Endpoint features are sharded host-side into an edge-ordered, transposed
    stream [x_src ; x_dst] so the device streams them at line rate (the
    SWDGE indirect-gather path measures ~8.7 ns/row on HW, which would
    dominate the kernel by >4x).
  - Both MLPs, log1p, biases, relu, and the aggregation all run on device in
    bf16 with fp32 PSUM accumulation.
"""
import numpy as np
import ml_dtypes

import concourse.bass as bass
import concourse.mybir as mybir
import concourse.tile as tile
from concourse import bacc
from concourse.bass_utils import run_bass_kernel_spmd

N_NODES = 100000
D_NODE = 64
D_EDGE = 4
H_MSG = 128
H_UPD = 128
N_CORES = 8
N_CORE = N_NODES // N_CORES          # 12500
BLK = 128
N_BLOCKS = (N_CORE + BLK - 1) // BLK  # 98
N_CORE_PAD = N_BLOCKS * BLK           # 12544
P = 128
ST = 4                                # chunks per supertile (512 edges)

BF16 = mybir.dt.bfloat16
F32 = mybir.dt.float32
nbf16 = ml_dtypes.bfloat16


def _build_program(chunk_meta, C, E_pad):
    """chunk_meta: list of (block_id, is_first_in_block, is_last_in_block)."""
    nc = bacc.Bacc("TRN2", target_bir_lowering=False, debug=False)

    xsdt = nc.declare_dram_parameter("xsdt", [P, E_pad], BF16, isOutput=False)
    eft = nc.declare_dram_parameter("eft", [D_EDGE, E_pad], F32, isOutput=False)
    xt = nc.declare_dram_parameter("xt", [D_NODE, N_CORE_PAD], BF16, isOutput=False)
    dstloc = nc.declare_dram_parameter("dstloc", [P, C], F32, isOutput=False)
    w1ab = nc.declare_dram_parameter("w1ab", [P, H_MSG], BF16, isOutput=False)
    w1c = nc.declare_dram_parameter("w1c", [D_EDGE, H_MSG], BF16, isOutput=False)
    w2 = nc.declare_dram_parameter("w2", [H_MSG, D_NODE], BF16, isOutput=False)
    u1 = nc.declare_dram_parameter("u1", [P, H_UPD], BF16, isOutput=False)
    u2 = nc.declare_dram_parameter("u2", [H_UPD, D_NODE], BF16, isOutput=False)
    b1c = nc.declare_dram_parameter("b1c", [H_MSG, 1], F32, isOutput=False)
    b2r = nc.declare_dram_parameter("b2r", [P, D_NODE], F32, isOutput=False)
    bu1c = nc.declare_dram_parameter("bu1c", [H_UPD, 1], F32, isOutput=False)
    bu2c = nc.declare_dram_parameter("bu2c", [D_NODE, 1], F32, isOutput=False)
    iota = nc.declare_dram_parameter("iota", [P, P], F32, isOutput=False)
    outt = nc.declare_dram_parameter("outt", [D_NODE, N_CORE_PAD], F32, isOutput=True)

    n_st = C // ST
    with tile.TileContext(nc) as tc:
        with (
            tc.tile_pool(name="const", bufs=1) as cpool,
            tc.tile_pool(name="xsd", bufs=3) as xsd_pool,
            tc.tile_pool(name="ef", bufs=2) as ef_pool,
            tc.tile_pool(name="hh", bufs=2) as h_pool,
            tc.tile_pool(name="msg", bufs=3) as msg_pool,
            tc.tile_pool(name="sel", bufs=3) as a_pool,
            tc.tile_pool(name="upd", bufs=2) as upd_pool,
            tc.tile_pool(name="phe", bufs=2, space="PSUM") as phe_pool,
            tc.tile_pool(name="pm", bufs=2, space="PSUM") as pm_pool,
            tc.tile_pool(name="pagg", bufs=2, space="PSUM") as pagg_pool,
            tc.tile_pool(name="pupd", bufs=1, space="PSUM") as pupd_pool,
        ):
            w1ab_sb = cpool.tile([P, H_MSG], BF16)
            nc.sync.dma_start(out=w1ab_sb[:], in_=w1ab[:])
            w1c_sb = cpool.tile([D_EDGE, H_MSG], BF16)
            nc.sync.dma_start(out=w1c_sb[:], in_=w1c[:])
            w2_sb = cpool.tile([H_MSG, D_NODE], BF16)
            nc.sync.dma_start(out=w2_sb[:], in_=w2[:])
            u1_sb = cpool.tile([P, H_UPD], BF16)
            nc.sync.dma_start(out=u1_sb[:], in_=u1[:])
            u2_sb = cpool.tile([H_UPD, D_NODE], BF16)
            nc.sync.dma_start(out=u2_sb[:], in_=u2[:])
            b1_sb = cpool.tile([H_MSG, 1], F32)
            nc.sync.dma_start(out=b1_sb[:], in_=b1c[:])
            b2_sb = cpool.tile([P, D_NODE], F32)
            nc.sync.dma_start(out=b2_sb[:], in_=b2r[:])
            bu1_sb = cpool.tile([H_UPD, 1], F32)
            nc.sync.dma_start(out=bu1_sb[:], in_=bu1c[:])
            bu2_sb = cpool.tile([D_NODE, 1], F32)
            nc.sync.dma_start(out=bu2_sb[:], in_=bu2c[:])
            iota_sb = cpool.tile([P, P], F32)
            nc.sync.dma_start(out=iota_sb[:], in_=iota[:])
            dl_sb = cpool.tile([P, C], F32)
            nc.sync.dma_start(out=dl_sb[:], in_=dstloc[:])

            p_agg = None
            for st_i in range(n_st):
                e0 = st_i * ST * P
                w = ST * P
                xsd_sb = xsd_pool.tile([P, w], BF16, tag="xsd")
                nc.sync.dma_start(out=xsd_sb[:], in_=xsdt[:, e0:e0 + w])
                ef_sb = ef_pool.tile([D_EDGE, w], F32, tag="ef32")
                nc.sync.dma_start(out=ef_sb[:], in_=eft[:, e0:e0 + w])
                efl_sb = ef_pool.tile([D_EDGE, w], BF16, tag="efb")
                # log1p(x) = ln(x + 1)
                nc.scalar.activation(
                    out=efl_sb[:], in_=ef_sb[:],
                    func=mybir.ActivationFunctionType.Ln, bias=1.0,
                )
                p_he = phe_pool.tile([H_MSG, w], F32, space="PSUM")
                nc.tensor.matmul(out=p_he[:], lhsT=w1ab_sb[:], rhs=xsd_sb[:],
                                 start=True, stop=False)
                nc.tensor.matmul(out=p_he[:], lhsT=w1c_sb[:], rhs=efl_sb[:],
                                 start=False, stop=True)
                h_sb = h_pool.tile([H_MSG, w], BF16, tag="h")
                # relu(psum + b1): per-partition bias add then max(0)
                nc.vector.tensor_scalar(
                    out=h_sb[:], in0=p_he[:], scalar1=b1_sb[:, :1], scalar2=0.0,
                    op0=mybir.AluOpType.add, op1=mybir.AluOpType.max,
                )
                for k in range(ST):
                    c = st_i * ST + k
                    blk_id, first, last = chunk_meta[c]
                    p_m = pm_pool.tile([P, D_NODE], F32, space="PSUM")
                    nc.tensor.matmul(out=p_m[:], lhsT=h_sb[:, k * P:(k + 1) * P],
                                     rhs=w2_sb[:], start=True, stop=True)
                    msg_sb = msg_pool.tile([P, D_NODE], BF16, tag="msg")
                    nc.vector.tensor_tensor(out=msg_sb[:], in0=p_m[:], in1=b2_sb[:],
                                            op=mybir.AluOpType.add)
                    a_sb = a_pool.tile([P, P], BF16, tag="A")
                    nc.vector.tensor_scalar(
                        out=a_sb[:], in0=iota_sb[:], scalar1=dl_sb[:, c:c + 1],
                        scalar2=None, op0=mybir.AluOpType.is_equal,
                    )
                    if first:
                        p_agg = pagg_pool.tile([D_NODE, P], F32, space="PSUM")
                    nc.tensor.matmul(out=p_agg[:], lhsT=msg_sb[:], rhs=a_sb[:],
                                     start=first, stop=last)
                    if last:
                        updt = upd_pool.tile([P, P], BF16, tag="updt")
                        nc.sync.dma_start(
                            out=updt[0:D_NODE, :],
                            in_=xt[:, blk_id * BLK:(blk_id + 1) * BLK])
                        nc.vector.tensor_copy(out=updt[D_NODE:P, :], in_=p_agg[:])
                        p_uh = pupd_pool.tile([H_UPD, P], F32, space="PSUM",
                                              tag="puh")
                        nc.tensor.matmul(out=p_uh[:], lhsT=u1_sb[:], rhs=updt[:],
                                         start=True, stop=True)
                        ru = upd_pool.tile([H_UPD, P], BF16, tag="ru")
                        nc.vector.tensor_scalar(
                            out=ru[:], in0=p_uh[:], scalar1=bu1_sb[:, :1],
                            scalar2=0.0,
                            op0=mybir.AluOpType.add, op1=mybir.AluOpType.max,
                        )
                        p_o = pupd_pool.tile([D_NODE, P], F32, space="PSUM",
                                             tag="po")
                        nc.tensor.matmul(out=p_o[:], lhsT=u2_sb[:], rhs=ru[:],
                                         start=True, stop=True)
                        o_sb = upd_pool.tile([D_NODE, P], F32, tag="osb")
                        nc.vector.tensor_scalar(
                            out=o_sb[:], in0=p_o[:], scalar1=bu2_sb[:, :1],
                            scalar2=None, op0=mybir.AluOpType.add,
                        )
                        nc.sync.dma_start(
                            out=outt[:, blk_id * BLK:(blk_id + 1) * BLK],
                            in_=o_sb[:])
    if not nc.is_finalized():
        nc.finalize()
    return nc


def kernel(x, edge_index, edge_features, W1, b1, W2, b2, U1, bu1, U2, bu2):
    x = np.asarray(x, dtype=np.float32)
    ei = np.asarray(edge_index).astype(np.int64)
    ef = np.asarray(edge_features, dtype=np.float32)
    src, dst = ei[0], ei[1]
    E = src.shape[0]

    order = np.argsort(dst, kind="stable")
    src_s, dst_s, ef_s = src[order], dst[order], ef[order]

    core_of = dst_s // N_CORE
    blk_of = (dst_s % N_CORE) // BLK

    # per-(core, block) edge counts -> shared chunk schedule
    cnt = np.zeros((N_CORES, N_BLOCKS), dtype=np.int64)
    np.add.at(cnt, (core_of, blk_of), 1)
    NB = np.maximum(1, (np.max(cnt, axis=0) + P - 1) // P)  # chunks per block
    pad4 = (-NB.sum()) % ST
    NB[-1] += pad4
    C = int(NB.sum())
    E_pad = C * P
    blk_chunk0 = np.concatenate([[0], np.cumsum(NB)[:-1]])

    chunk_meta = []
    for b in range(N_BLOCKS):
        for j in range(int(NB[b])):
            chunk_meta.append((b, j == 0, j == int(NB[b]) - 1))

    xbf = x.astype(nbf16)
    w1ab_h = np.ascontiguousarray(W1[:2 * D_NODE]).astype(nbf16)
    w1c_h = np.ascontiguousarray(W1[2 * D_NODE:]).astype(nbf16)
    w2_h = np.asarray(W2).astype(nbf16)
    u1_h = np.asarray(U1).astype(nbf16)
    u2_h = np.asarray(U2).astype(nbf16)
    b1_h = np.asarray(b1, dtype=np.float32).reshape(H_MSG, 1)
    b2_h = np.tile(np.asarray(b2, dtype=np.float32), (P, 1))
    bu1_h = np.asarray(bu1, dtype=np.float32).reshape(H_UPD, 1)
    bu2_h = np.asarray(bu2, dtype=np.float32).reshape(D_NODE, 1)
    iota_h = np.ascontiguousarray(np.tile(np.arange(P, dtype=np.float32), (P, 1)))

    # per-core edge slot assignment (vectorized): edge -> padded slot index
    in_maps = []
    for c in range(N_CORES):
        m = core_of == c
        eb = blk_of[m]
        # edges are dst-sorted, so eb is sorted; rank within block =
        # position - first position of that block
        first_pos = np.searchsorted(eb, np.arange(N_BLOCKS), side="left")
        rank = np.arange(eb.shape[0]) - first_pos[eb]
        slot = (blk_chunk0[eb] * P + rank).astype(np.int64)

        e_src = src_s[m]
        e_dst = dst_s[m]
        e_ef = ef_s[m]

        xsdt_h = np.zeros((E_pad, 2 * D_NODE), dtype=nbf16)
        xsdt_h[slot, :D_NODE] = xbf[e_src]
        xsdt_h[slot, D_NODE:] = xbf[e_dst]
        xsdt_h = np.ascontiguousarray(xsdt_h.T)

        eft_h = np.zeros((E_pad, D_EDGE), dtype=np.float32)
        eft_h[slot] = e_ef
        eft_h = np.ascontiguousarray(eft_h.T)

        dl = np.full(E_pad, -1.0, dtype=np.float32)
        dl[slot] = (e_dst % N_CORE) % BLK
        dstloc_h = np.ascontiguousarray(dl.reshape(C, P).T)

        xt_h = np.zeros((N_CORE_PAD, D_NODE), dtype=nbf16)
        xt_h[:N_CORE] = xbf[c * N_CORE:(c + 1) * N_CORE]
        xt_h = np.ascontiguousarray(xt_h.T)

        in_maps.append({
            "xsdt": xsdt_h, "eft": eft_h, "xt": xt_h, "dstloc": dstloc_h,
            "w1ab": w1ab_h, "w1c": w1c_h, "w2": w2_h, "u1": u1_h, "u2": u2_h,
            "b1c": b1_h, "b2r": b2_h, "bu1c": bu1_h, "bu2c": bu2_h,
            "iota": iota_h,
        })

    nc = _build_program(chunk_meta, C, E_pad)
    res = run_bass_kernel_spmd(nc, in_maps, list(range(N_CORES)))
    global _last_results
    _last_results = res

    out = np.empty((N_NODES, D_NODE), dtype=np.float32)
    for c in range(N_CORES):
        out[c * N_CORE:(c + 1) * N_CORE] = res.results[c]["outt"].T[:N_CORE]
    return out
